# revision 33
# baseline (speedup 1.0000x reference)
"""Block-dequant linear kernel for TRN2 (8 NeuronCores).

Computes y = x @ (weight_q * block_scale).T with
  x:        [64, 7168]  f32
  weight_q: [18432, 7168] f32 (block-quantized codes)
  scale:    [144, 56]   f32 (one scale per 128x128 block)

Sharding: row-parallel over out_features. Each of the 8 cores gets a
[2304, 7168] slice of weight_q and an [18, 56] slice of scale; x is
replicated; per-core outputs y_c = [64, 2304] are concatenated on host.

Host-side layout prep (no FLOPs on W beyond dtype rounding):
  - W slice is pre-transposed to wT [7168, 2304] so the device DMAs
    contraction-major strips directly; no on-chip PE transposes.
  - x is pre-interleaved to xt[p, ib*64+t] = x[t, ib*128+p] so the
    whole stationary operand arrives in one contiguous DMA.
  - scale rows are pre-broadcast to all 128 partitions.
  - W and x are cast to fp16 on host, halving HBM traffic and
    doubling PE/DVE throughput (tolerance is 2e-2; fp16 lands ~3e-4).

Scheme "f16p" (default): the dequant scale is folded into the matmul's
STATIONARY operand. tokens=64 <= 128/2, so each [128,128] stationary
packs TWO scale variants of xt_ib: cols 0:64 scaled by s[2j, ib], cols
64:128 by s[2j+1, ib]. A matmul against the raw W strip columns of
o-blocks (2j, 2j+1) (N=256) then accumulates the correctly scaled
product in the [0:64, 0:128] and [64:128, 128:256] PSUM quadrants (the
off-diagonal quadrants accumulate garbage that is never read). The
9 scaled-x tables are built by 9 large DVE ops in the prologue
(8.3M elems, half of W's 16.5M), and the per-strip hot path is pure
DMA -> matmul with no dequant stage at all.

Scheme "f16": per-strip DVE dequant (slow-mode broadcast multiply),
then 5 N<=512 matmuls per strip.
"""

import numpy as np

import concourse.bass as bass  # noqa: E402
from concourse import bacc  # noqa: E402
import concourse.mybir as mybir  # noqa: E402
import concourse.tile as tile  # noqa: E402
from concourse.bass_utils import run_bass_kernel_spmd  # noqa: E402

TOKENS = 64
IN_F = 7168
OUT_F = 18432
N_CORES = 8
O_PER = OUT_F // N_CORES  # 2304
OB = O_PER // 128  # 18 o-blocks per core
NP_PAIR = OB // 2  # 9 ob pairs
IBC = IN_F // 128  # 56 i-blocks
CHUNKS = [(0, 512), (512, 512), (1024, 512), (1536, 512), (2048, 256)]

_DT = {
    "f32r": (mybir.dt.float32r, np.float32),
    "f32": (mybir.dt.float32, np.float32),
    "f16": (mybir.dt.float16, np.float16),
    "f16p": (mybir.dt.float16, np.float16),
}
try:
    import ml_dtypes

    _DT["bf16"] = (mybir.dt.bfloat16, ml_dtypes.bfloat16)
except ImportError:
    pass


def _build_strip_scheme(nc, tc, dt_mode, xt_h, wt_h, sb_h, y_h):
    """Per-strip DVE dequant + 5 chunk matmuls."""
    f32 = mybir.dt.float32
    mm_dt, _ = _DT[dt_mode]
    esize = 4 if dt_mode in ("f32r", "f32") else 2
    with (
        tc.tile_pool(name="const", bufs=1) as cpool,
        tc.tile_pool(name="wraw", bufs=4 if esize == 4 else 6) as wraw,
        tc.tile_pool(name="wdq", bufs=4) as wdq,
        tc.tile_pool(name="opool", bufs=2) as opool,
        tc.tile_pool(name="accp", bufs=1, space="PSUM") as accp,
    ):
        xt = cpool.tile([128, IBC * TOKENS], mm_dt, name="xt")
        nc.sync.dma_start(out=xt[:, :], in_=xt_h[:, :])
        s_b = cpool.tile([128, OB * IBC], f32, name="sb")
        nc.sync.dma_start(out=s_b[:, :], in_=sb_h[:, :])
        s_b3 = s_b[:, :].rearrange("p (ob ib) -> p ob ib", ib=IBC)

        accs = [
            accp.tile([TOKENS, 512], f32, name=f"acc{k}")[:, :ch]
            for k, (_, ch) in enumerate(CHUNKS)
        ]
        for ib in range(IBC):
            wr = wraw.tile([128, O_PER], mm_dt, tag="wr", name="wr")
            nc.sync.dma_start(out=wr[:, :], in_=wt_h[ib * 128 : (ib + 1) * 128, :])
            wd = wdq.tile([128, O_PER], mm_dt, tag="wd", name="wd")
            sca = s_b3[:, :, ib].unsqueeze(2).broadcast_to([128, OB, 128])
            in0 = wr[:, :]
            if dt_mode == "f32r":
                in0 = in0.bitcast(f32)
            nc.vector.tensor_mul(
                out=wd[:, :].rearrange("p (ob c) -> p ob c", c=128),
                in0=in0.rearrange("p (ob c) -> p ob c", c=128),
                in1=sca,
            )
            for k, (cb, ch) in enumerate(CHUNKS):
                nc.tensor.matmul(
                    accs[k],
                    lhsT=xt[:, ib * TOKENS : (ib + 1) * TOKENS],
                    rhs=wd[:, cb : cb + ch],
                    start=(ib == 0),
                    stop=(ib == IBC - 1),
                )
        for k, (cb, ch) in enumerate(CHUNKS):
            ysb = opool.tile([TOKENS, 512], f32, tag="ysb", name="ysb")[:, :ch]
            nc.any.tensor_copy(out=ysb, in_=accs[k])
            nc.sync.dma_start(out=y_h[:, cb : cb + ch], in_=ysb)


def _build_pair_scheme(nc, tc, xt_h, wt_h, sb_h, y_h):
    """Scale folded into paired stationary tables; raw-W matmuls."""
    f32 = mybir.dt.float32
    f16 = mybir.dt.float16
    with (
        tc.tile_pool(name="const", bufs=1) as cpool,
        tc.tile_pool(name="wraw", bufs=8) as wraw,
        tc.tile_pool(name="xsp", bufs=2) as xsp,
        tc.tile_pool(name="opool", bufs=1) as opool,
        tc.tile_pool(name="accp", bufs=1, space="PSUM") as accp,
    ):
        NQ = 7
        QIB = IBC // NQ  # 8 i-blocks per table batch

        # xt/sb issue on the ACT hardware DGE so they don't queue behind
        # the W-strip issue path on Sync; sb is fp16 to land faster
        s_b = cpool.tile([128, OB * IBC], f16, name="sb")
        nc.scalar.dma_start(out=s_b[:, :], in_=sb_h[:, :])
        s_b3 = s_b[:, :].rearrange("p (ob ib) -> p ob ib", ib=IBC)
        xt = cpool.tile([128, IBC * TOKENS], f16, name="xt")
        nc.scalar.dma_start(out=xt[:, :], in_=xt_h[:, :])

        # 9 accumulators [128, 256] packed two per PSUM bank
        banks = [accp.tile([128, 512], f32, name=f"bank{b}") for b in range(5)]
        accs = [banks[j // 2][:, (j % 2) * 256 : (j % 2) * 256 + 256] for j in range(NP_PAIR)]

        # scaled-x tables xs_j[p, ib*128 + v*64 + t] = xt * s[2j+v, ib],
        # built just-in-time per ib-batch in a 2-deep ring (the data is
        # consumed streaming, freeing SBUF for deep W-strip prefetch)
        def emit_tables(q):
            i0 = q * QIB
            xt3 = (
                xt[:, q * QIB * TOKENS : (q + 1) * QIB * TOKENS]
                .rearrange("p (ib t) -> p ib t", t=TOKENS)
                .unsqueeze(2)
                .broadcast_to([128, QIB, 2, TOKENS])
            )
            xts = []
            for j in range(NP_PAIR):
                xs = xsp.tile([128, QIB * 128], f16, tag=f"xs{j}", name=f"xs{j}")
                sv = (
                    s_b3[:, 2 * j : 2 * j + 2, i0 : i0 + QIB]
                    .transpose([0, 2, 1])
                    .unsqueeze(3)
                    .broadcast_to([128, QIB, 2, TOKENS])
                )
                nc.vector.tensor_mul(
                    out=xs[:, :].rearrange(
                        "p (ib v t) -> p ib v t", v=2, t=TOKENS
                    ),
                    in0=xt3,
                    in1=sv,
                )
                xts.append(xs)
            return xts

        for q in range(NQ):
            xss = emit_tables(q)
            for kk in range(QIB // 2):
                # one DMA carries TWO i-block strips (rows r and r+128 of
                # the wt slab land in partition r%128, col-halves) to
                # halve the dma_start issue load on the Sync sequencer
                ib0 = q * QIB + 2 * kk
                wr = wraw.tile([128, 2 * O_PER], f16, tag="wr", name="wr")
                nc.sync.dma_start(
                    out=wr[:, :].rearrange("p (h o) -> p h o", h=2),
                    in_=wt_h[ib0 * 128 : (ib0 + 2) * 128, :].rearrange(
                        "(h p) o -> p h o", h=2
                    ),
                )
                for h in range(2):
                    ib = ib0 + h
                    k = 2 * kk + h
                    for j in range(NP_PAIR):
                        # start=True zeroes the WHOLE PSUM bank, so only
                        # the first slot of each shared bank may set it;
                        # the second slot accumulates onto the zeroed
                        # region.
                        nc.tensor.matmul(
                            accs[j],
                            lhsT=xss[j][:, k * 128 : (k + 1) * 128],
                            rhs=wr[:, h * O_PER + j * 256 : h * O_PER + (j + 1) * 256],
                            start=(ib == 0 and j % 2 == 0),
                            stop=(ib == IBC - 1),
                            skip_group_check=True,
                        )
        ysb = opool.tile([TOKENS, O_PER], f32, name="ysb")

        def evac(dst, src, on_act):
            # evacuations split DVE/ACT to halve the serial tail
            if on_act:
                nc.scalar.activation(
                    dst, src, mybir.ActivationFunctionType.Copy
                )
            else:
                nc.vector.tensor_copy(out=dst, in_=src)

        for j in range(NP_PAIR):
            evac(
                ysb[:, (2 * j) * 128 : (2 * j + 1) * 128],
                accs[j][0:TOKENS, 0:128],
                on_act=(j % 2 == 1),
            )
            evac(
                ysb[:, (2 * j + 1) * 128 : (2 * j + 2) * 128],
                accs[j][TOKENS:128, 128:256],
                on_act=(j % 2 == 0),
            )
            if j in (2, 5):
                c0, c1 = (0, 768) if j == 2 else (768, 1536)
                nc.sync.dma_start(out=y_h[:, c0:c1], in_=ysb[:, c0:c1])
        nc.sync.dma_start(out=y_h[:, 1536:], in_=ysb[:, 1536:])


def build_nc(dt_mode: str = "f16p") -> bass.Bass:
    f32 = mybir.dt.float32
    mm_dt, _ = _DT[dt_mode]
    nc = bacc.Bacc()
    sb_dt = mybir.dt.float16 if dt_mode == "f16p" else f32
    xt_h = nc.dram_tensor("xt", [128, IBC * TOKENS], mm_dt, kind="ExternalInput")
    wt_h = nc.dram_tensor("wt", [IN_F, O_PER], mm_dt, kind="ExternalInput")
    sb_h = nc.dram_tensor("sb", [128, OB * IBC], sb_dt, kind="ExternalInput")
    y_h = nc.dram_tensor("y", [TOKENS, O_PER], f32, kind="ExternalOutput")

    with tile.TileContext(nc) as tc:
        if dt_mode == "f16p":
            _build_pair_scheme(nc, tc, xt_h, wt_h, sb_h, y_h)
        else:
            _build_strip_scheme(nc, tc, dt_mode, xt_h, wt_h, sb_h, y_h)
    nc.compile()
    return nc


_NC_CACHE: dict = {}


def _get_nc(dt_mode="f16p"):
    if dt_mode not in _NC_CACHE:
        _NC_CACHE[dt_mode] = build_nc(dt_mode)
    return _NC_CACHE[dt_mode]


def kernel(x, weight_q, scale, _trace=False, _dt="f16p"):
    x = np.ascontiguousarray(np.asarray(x, dtype=np.float32))
    weight_q = np.asarray(weight_q, dtype=np.float32)
    scale = np.asarray(scale, dtype=np.float32)
    nc = _get_nc(_dt)
    _, np_dt = _DT[_dt]
    # xt[p, ib*64+t] = x[t, ib*128+p]
    xt = np.ascontiguousarray(
        x.reshape(TOKENS, IBC, 128).transpose(2, 1, 0).reshape(128, IBC * TOKENS)
    ).astype(np_dt)
    in_maps = []
    for c in range(N_CORES):
        wt = np.ascontiguousarray(
            weight_q[c * O_PER : (c + 1) * O_PER].astype(np_dt).T
        )
        s_row = scale[c * OB : (c + 1) * OB].reshape(1, OB * IBC)
        if _dt == "f16p":
            s_row = s_row.astype(np.float16)
        sb = np.ascontiguousarray(np.broadcast_to(s_row, (128, OB * IBC)))
        in_maps.append({"xt": xt, "wt": wt, "sb": sb})
    res = run_bass_kernel_spmd(nc, in_maps, list(range(N_CORES)), trace=_trace)
    y = np.concatenate([res.results[c]["y"] for c in range(N_CORES)], axis=1)
    if _trace:
        return y, res
    return y


if __name__ == "__main__":
    rng = np.random.default_rng(0)
    x = rng.standard_normal((TOKENS, IN_F), dtype=np.float32)
    w = rng.standard_normal((OUT_F, IN_F), dtype=np.float32)
    s = rng.random((OUT_F // 128, IN_F // 128), dtype=np.float32)
    y = kernel(x, w, s)
    print("ok", y.shape, y.dtype)


# revision 34
# speedup vs baseline: 1.0789x; 1.0789x over previous
"""Block-dequant linear kernel for TRN2 (8 NeuronCores).

Computes y = x @ (weight_q * block_scale).T with
  x:        [64, 7168]  f32
  weight_q: [18432, 7168] f32 (block-quantized codes)
  scale:    [144, 56]   f32 (one scale per 128x128 block)

Sharding: row-parallel over out_features. Each of the 8 cores gets a
[2304, 7168] slice of weight_q and an [18, 56] slice of scale; x is
replicated; per-core outputs y_c = [64, 2304] are concatenated on host.

Host-side layout prep (no FLOPs on W beyond dtype rounding):
  - W slice is pre-transposed to wT [7168, 2304] so the device DMAs
    contraction-major strips directly; no on-chip PE transposes.
  - x is pre-interleaved to xt[p, ib*64+t] = x[t, ib*128+p] so the
    whole stationary operand arrives in one contiguous DMA.
  - scale rows are pre-broadcast to all 128 partitions.
  - W and x are cast to fp16 on host, halving HBM traffic and
    doubling PE/DVE throughput (tolerance is 2e-2; fp16 lands ~3e-4).

Scheme "f16p" (default): the dequant scale is folded into the matmul's
STATIONARY operand. tokens=64 <= 128/2, so each [128,128] stationary
packs TWO scale variants of xt_ib: cols 0:64 scaled by s[2j, ib], cols
64:128 by s[2j+1, ib]. A matmul against the raw W strip columns of
o-blocks (2j, 2j+1) (N=256) then accumulates the correctly scaled
product in the [0:64, 0:128] and [64:128, 128:256] PSUM quadrants (the
off-diagonal quadrants accumulate garbage that is never read). The
9 scaled-x tables are built by 9 large DVE ops in the prologue
(8.3M elems, half of W's 16.5M), and the per-strip hot path is pure
DMA -> matmul with no dequant stage at all.

Scheme "f16": per-strip DVE dequant (slow-mode broadcast multiply),
then 5 N<=512 matmuls per strip.
"""

import numpy as np

import concourse.bass as bass  # noqa: E402
from concourse import bacc  # noqa: E402
import concourse.mybir as mybir  # noqa: E402
import concourse.tile as tile  # noqa: E402
from concourse.bass_utils import run_bass_kernel_spmd  # noqa: E402

TOKENS = 64
IN_F = 7168
OUT_F = 18432
N_CORES = 8
O_PER = OUT_F // N_CORES  # 2304
OB = O_PER // 128  # 18 o-blocks per core
NP_PAIR = OB // 2  # 9 ob pairs
IBC = IN_F // 128  # 56 i-blocks
CHUNKS = [(0, 512), (512, 512), (1024, 512), (1536, 512), (2048, 256)]

_DT = {
    "f32r": (mybir.dt.float32r, np.float32),
    "f32": (mybir.dt.float32, np.float32),
    "f16": (mybir.dt.float16, np.float16),
    "f16p": (mybir.dt.float16, np.float16),
}
try:
    import ml_dtypes

    _DT["bf16"] = (mybir.dt.bfloat16, ml_dtypes.bfloat16)
except ImportError:
    pass


def _build_strip_scheme(nc, tc, dt_mode, xt_h, wt_h, sb_h, y_h):
    """Per-strip DVE dequant + 5 chunk matmuls."""
    f32 = mybir.dt.float32
    mm_dt, _ = _DT[dt_mode]
    esize = 4 if dt_mode in ("f32r", "f32") else 2
    with (
        tc.tile_pool(name="const", bufs=1) as cpool,
        tc.tile_pool(name="wraw", bufs=4 if esize == 4 else 6) as wraw,
        tc.tile_pool(name="wdq", bufs=4) as wdq,
        tc.tile_pool(name="opool", bufs=2) as opool,
        tc.tile_pool(name="accp", bufs=1, space="PSUM") as accp,
    ):
        xt = cpool.tile([128, IBC * TOKENS], mm_dt, name="xt")
        nc.sync.dma_start(out=xt[:, :], in_=xt_h[:, :])
        s_b = cpool.tile([128, OB * IBC], f32, name="sb")
        nc.sync.dma_start(out=s_b[:, :], in_=sb_h[:, :])
        s_b3 = s_b[:, :].rearrange("p (ob ib) -> p ob ib", ib=IBC)

        accs = [
            accp.tile([TOKENS, 512], f32, name=f"acc{k}")[:, :ch]
            for k, (_, ch) in enumerate(CHUNKS)
        ]
        for ib in range(IBC):
            wr = wraw.tile([128, O_PER], mm_dt, tag="wr", name="wr")
            nc.sync.dma_start(out=wr[:, :], in_=wt_h[ib * 128 : (ib + 1) * 128, :])
            wd = wdq.tile([128, O_PER], mm_dt, tag="wd", name="wd")
            sca = s_b3[:, :, ib].unsqueeze(2).broadcast_to([128, OB, 128])
            in0 = wr[:, :]
            if dt_mode == "f32r":
                in0 = in0.bitcast(f32)
            nc.vector.tensor_mul(
                out=wd[:, :].rearrange("p (ob c) -> p ob c", c=128),
                in0=in0.rearrange("p (ob c) -> p ob c", c=128),
                in1=sca,
            )
            for k, (cb, ch) in enumerate(CHUNKS):
                nc.tensor.matmul(
                    accs[k],
                    lhsT=xt[:, ib * TOKENS : (ib + 1) * TOKENS],
                    rhs=wd[:, cb : cb + ch],
                    start=(ib == 0),
                    stop=(ib == IBC - 1),
                )
        for k, (cb, ch) in enumerate(CHUNKS):
            ysb = opool.tile([TOKENS, 512], f32, tag="ysb", name="ysb")[:, :ch]
            nc.any.tensor_copy(out=ysb, in_=accs[k])
            nc.sync.dma_start(out=y_h[:, cb : cb + ch], in_=ysb)


def _build_pair_scheme(nc, tc, xt_h, wt_h, sb_h, y_h):
    """Scale folded into paired stationary tables; raw-W matmuls."""
    f32 = mybir.dt.float32
    f16 = mybir.dt.float16
    with (
        tc.tile_pool(name="const", bufs=1) as cpool,
        tc.tile_pool(name="wraw", bufs=8) as wraw,
        tc.tile_pool(name="xsp", bufs=2) as xsp,
        tc.tile_pool(name="opool", bufs=1) as opool,
        tc.tile_pool(name="accp", bufs=1, space="PSUM") as accp,
    ):
        NQ = 7
        QIB = IBC // NQ  # 8 i-blocks per table batch

        # xt/sb issue on the ACT hardware DGE so they don't queue behind
        # the W-strip issue path on Sync; sb is fp16 to land faster
        s_b = cpool.tile([128, OB * IBC], f16, name="sb")
        nc.scalar.dma_start(out=s_b[:, :], in_=sb_h[:, :])
        s_b3 = s_b[:, :].rearrange("p (ob ib) -> p ob ib", ib=IBC)
        xt = cpool.tile([128, IBC * TOKENS], f16, name="xt")
        nc.scalar.dma_start(out=xt[:, :], in_=xt_h[:, :])

        # 9 accumulators [128, 256] packed two per PSUM bank
        banks = [accp.tile([128, 512], f32, name=f"bank{b}") for b in range(5)]
        accs = [banks[j // 2][:, (j % 2) * 256 : (j % 2) * 256 + 256] for j in range(NP_PAIR)]

        # scaled-x tables xs_j[p, ib*128 + v*64 + t] = xt * s[2j+v, ib],
        # built just-in-time per ib-batch in a 2-deep ring (the data is
        # consumed streaming, freeing SBUF for deep W-strip prefetch)
        def emit_tables(q):
            i0 = q * QIB
            xt3 = (
                xt[:, q * QIB * TOKENS : (q + 1) * QIB * TOKENS]
                .rearrange("p (ib t) -> p ib t", t=TOKENS)
                .unsqueeze(2)
                .broadcast_to([128, QIB, 2, TOKENS])
            )
            xts = []
            for j in range(NP_PAIR):
                xs = xsp.tile([128, QIB * 128], f16, tag=f"xs{j}", name=f"xs{j}")
                sv = (
                    s_b3[:, 2 * j : 2 * j + 2, i0 : i0 + QIB]
                    .transpose([0, 2, 1])
                    .unsqueeze(3)
                    .broadcast_to([128, QIB, 2, TOKENS])
                )
                nc.vector.tensor_mul(
                    out=xs[:, :].rearrange(
                        "p (ib v t) -> p ib v t", v=2, t=TOKENS
                    ),
                    in0=xt3,
                    in1=sv,
                )
                xts.append(xs)
            return xts

        for q in range(NQ):
            xss = emit_tables(q)
            for kk in range(QIB // 2):
                # one DMA carries TWO i-block strips (rows r and r+128 of
                # the wt slab land in partition r%128, col-halves) to
                # halve the dma_start issue load on the Sync sequencer
                ib0 = q * QIB + 2 * kk
                wr = wraw.tile([128, 2 * O_PER], f16, tag="wr", name="wr")
                dge = nc.sync if kk % 2 == 0 else nc.scalar
                dge.dma_start(
                    out=wr[:, :].rearrange("p (h o) -> p h o", h=2),
                    in_=wt_h[ib0 * 128 : (ib0 + 2) * 128, :].rearrange(
                        "(h p) o -> p h o", h=2
                    ),
                )
                for h in range(2):
                    ib = ib0 + h
                    k = 2 * kk + h
                    for j in range(NP_PAIR):
                        # start=True zeroes the WHOLE PSUM bank, so only
                        # the first slot of each shared bank may set it;
                        # the second slot accumulates onto the zeroed
                        # region.
                        nc.tensor.matmul(
                            accs[j],
                            lhsT=xss[j][:, k * 128 : (k + 1) * 128],
                            rhs=wr[:, h * O_PER + j * 256 : h * O_PER + (j + 1) * 256],
                            start=(ib == 0 and j % 2 == 0),
                            stop=(ib == IBC - 1),
                            skip_group_check=True,
                        )
        ysb = opool.tile([TOKENS, O_PER], f32, name="ysb")

        def evac(dst, src, on_act):
            # evacuations split DVE/ACT to halve the serial tail
            if on_act:
                nc.scalar.activation(
                    dst, src, mybir.ActivationFunctionType.Copy
                )
            else:
                nc.vector.tensor_copy(out=dst, in_=src)

        for j in range(NP_PAIR):
            evac(
                ysb[:, (2 * j) * 128 : (2 * j + 1) * 128],
                accs[j][0:TOKENS, 0:128],
                on_act=(j % 2 == 1),
            )
            evac(
                ysb[:, (2 * j + 1) * 128 : (2 * j + 2) * 128],
                accs[j][TOKENS:128, 128:256],
                on_act=(j % 2 == 0),
            )
            if j in (2, 5):
                c0, c1 = (0, 768) if j == 2 else (768, 1536)
                nc.sync.dma_start(out=y_h[:, c0:c1], in_=ysb[:, c0:c1])
        nc.sync.dma_start(out=y_h[:, 1536:], in_=ysb[:, 1536:])


def build_nc(dt_mode: str = "f16p") -> bass.Bass:
    f32 = mybir.dt.float32
    mm_dt, _ = _DT[dt_mode]
    nc = bacc.Bacc()
    sb_dt = mybir.dt.float16 if dt_mode == "f16p" else f32
    xt_h = nc.dram_tensor("xt", [128, IBC * TOKENS], mm_dt, kind="ExternalInput")
    wt_h = nc.dram_tensor("wt", [IN_F, O_PER], mm_dt, kind="ExternalInput")
    sb_h = nc.dram_tensor("sb", [128, OB * IBC], sb_dt, kind="ExternalInput")
    y_h = nc.dram_tensor("y", [TOKENS, O_PER], f32, kind="ExternalOutput")

    with tile.TileContext(nc) as tc:
        if dt_mode == "f16p":
            _build_pair_scheme(nc, tc, xt_h, wt_h, sb_h, y_h)
        else:
            _build_strip_scheme(nc, tc, dt_mode, xt_h, wt_h, sb_h, y_h)
    nc.compile()
    return nc


_NC_CACHE: dict = {}


def _get_nc(dt_mode="f16p"):
    if dt_mode not in _NC_CACHE:
        _NC_CACHE[dt_mode] = build_nc(dt_mode)
    return _NC_CACHE[dt_mode]


def kernel(x, weight_q, scale, _trace=False, _dt="f16p"):
    x = np.ascontiguousarray(np.asarray(x, dtype=np.float32))
    weight_q = np.asarray(weight_q, dtype=np.float32)
    scale = np.asarray(scale, dtype=np.float32)
    nc = _get_nc(_dt)
    _, np_dt = _DT[_dt]
    # xt[p, ib*64+t] = x[t, ib*128+p]
    xt = np.ascontiguousarray(
        x.reshape(TOKENS, IBC, 128).transpose(2, 1, 0).reshape(128, IBC * TOKENS)
    ).astype(np_dt)
    in_maps = []
    for c in range(N_CORES):
        wt = np.ascontiguousarray(
            weight_q[c * O_PER : (c + 1) * O_PER].astype(np_dt).T
        )
        s_row = scale[c * OB : (c + 1) * OB].reshape(1, OB * IBC)
        if _dt == "f16p":
            s_row = s_row.astype(np.float16)
        sb = np.ascontiguousarray(np.broadcast_to(s_row, (128, OB * IBC)))
        in_maps.append({"xt": xt, "wt": wt, "sb": sb})
    res = run_bass_kernel_spmd(nc, in_maps, list(range(N_CORES)), trace=_trace)
    y = np.concatenate([res.results[c]["y"] for c in range(N_CORES)], axis=1)
    if _trace:
        return y, res
    return y


if __name__ == "__main__":
    rng = np.random.default_rng(0)
    x = rng.standard_normal((TOKENS, IN_F), dtype=np.float32)
    w = rng.standard_normal((OUT_F, IN_F), dtype=np.float32)
    s = rng.random((OUT_F // 128, IN_F // 128), dtype=np.float32)
    y = kernel(x, w, s)
    print("ok", y.shape, y.dtype)
